# Initial kernel scaffold
#
"""GAT 2-layer + global mean pool, distributed over 8 TRN2 NeuronCores.

Sharding: 8 dst-shards of 6250 nodes. Per layer:
  - each core computes rotated features h' = x @ (W @ Q) for its shard
    (bf16), where Q is orthonormal with q0 ~ a_src and span(q0,q1) covering
    a_dst, so asrc = s0*h'[0] and adst = alpha*h'[0] + beta*h'[1] are free
    columns of every gathered row; AllGather -> full 50016-row table
  - edges split into two passes by src half (int16 gather idx < 32767)
  - per pass: degree-sorted windows of 128 dst nodes, padded rounds;
    dma_gather (4096 idxs/call, single_packet=False) pulls h'[src] rows;
    ACT computes z = s0*G[:,:,0] + adst (per-window bias), DVE lrelu,
    ACT exp -> w; one batched DVE mul scales rows by w;
    PE accumulates numerators per window via identity-lhsT matmuls (PSUM),
    DVE reduces w for denominators;
    dma_scatter_add canonicalizes [numer'|denom] into a DRAM partial buffer
  - normalize: un-rotate numer = numer' @ Q^T (PE), reciprocal, bias, relu;
    one-hot matmul pooling; final AllReduce of pooled embeddings.
"""

import os
import sys

import numpy as np

# ---------------- constants (hardcoded problem shape) ----------------
N = 50000
E = 600000
G = 1024
D = 128
NEG_SLOPE = 0.2
NC = 8
SH = N // NC          # 6250 nodes per shard
HB = N // 2           # 25000 per src half
BLK = SH + 2          # block rows in table: shard + zero row + pad row
TBL = NC * BLK        # 50016
HALF_ROWS = 4 * BLK   # 25008 rows per half
NWIN = (SH + 127) // 128   # 49 windows
WSLOT = NWIN * 128         # 6272 slots
ZERO_REL = SH              # zero row, relative to block 0 of a half
PAD_REL = SH + 1           # pad row (h = -C*a_src)
NB = 32                    # rounds per gather call (single_packet=False lifts the 1024-idx limit)
WGRP = 7                   # windows per scatter group
PAD_C = 30000.0            # pad row scale -> logit ~ -30000*||a||^2
DUMMY_NODE = SH            # scatter dummy local row
EMB_ROWS = 2176            # 2*1024 + dummy + pad to 128 multiple
EMB_DUMMY = 2048

_CACHE = {}


def _table_row(n):
    return (n // SH) * BLK + (n % SH)


def _wrap16(a):
    """int array -> [128, len/16] int16 wrapped + replicated layout."""
    a = np.asarray(a, dtype=np.int16)
    L = a.shape[0]
    assert L % 16 == 0
    w = a.reshape(L // 16, 16).T  # [16, L/16]
    return np.tile(w, (8, 1)).copy()  # [128, L/16]


def host_prep(edge_index, batch):
    """Pure index preprocessing. Returns per-core input dicts pieces + meta."""
    src = np.concatenate([np.asarray(edge_index[0]), np.arange(N)]).astype(np.int64)
    dst = np.concatenate([np.asarray(edge_index[1]), np.arange(N)]).astype(np.int64)
    batch = np.asarray(batch).astype(np.int64)

    trow = _table_row(np.arange(N))  # table row of each node

    cores = []
    for c in range(NC):
        sel = (dst // SH) == c
        s_c = src[sel]
        d_c = dst[sel] - c * SH
        passes = []
        for pi in (0, 1):
            m = (s_c // HB) == pi
            s = s_c[m]
            dl = d_c[m]
            cnt = np.bincount(dl, minlength=SH)
            # CSR by local dst
            eorder = np.argsort(dl, kind="stable")
            s_sorted = s[eorder]
            starts = np.zeros(SH + 1, dtype=np.int64)
            starts[1:] = np.cumsum(cnt)
            perm = np.argsort(-cnt, kind="stable")  # nodes by degree desc
            rw = np.zeros(NWIN, dtype=np.int64)
            for w in range(NWIN):
                nodes = perm[w * 128:(w + 1) * 128]
                if len(nodes):
                    rw[w] = cnt[nodes].max()
            passes.append(dict(cnt=cnt, s_sorted=s_sorted, starts=starts,
                               perm=perm, rw=rw))
        cores.append(passes)

    # uniform window profiles across cores (per pass)
    RW = []
    for pi in (0, 1):
        rw = np.zeros(NWIN, dtype=np.int64)
        for c in range(NC):
            rw = np.maximum(rw, cores[c][pi]["rw"])
        RW.append(rw)
    NR = [int(RW[0].sum()), int(RW[1].sum())]

    # per-core index arrays
    percore = []
    for c in range(NC):
        d = {}
        for pi in (0, 1):
            P = cores[c][pi]
            perm, cnt, starts, s_sorted = P["perm"], P["cnt"], P["starts"], P["s_sorted"]
            half_base = pi * HALF_ROWS
            # node grid [NWIN, 128], -1 for pad slots
            grid = np.full((NWIN, 128), -1, dtype=np.int64)
            flat = perm
            grid.reshape(-1)[: len(flat)] = flat
            # round-0 self-gather idx, both halves
            r0a = np.full(WSLOT, ZERO_REL, dtype=np.int64)
            r0b = np.full(WSLOT, ZERO_REL, dtype=np.int64)
            gv = grid.reshape(-1)
            valid = gv >= 0
            gl = np.where(valid, gv + c * SH, 0)   # canonical node id
            isA = valid & (gl // HB == 0)
            isB = valid & (gl // HB == 1)
            r0a[isA] = trow[gl[isA]]
            r0b[isB] = trow[gl[isB]] - HALF_ROWS
            d[f"r0a_p{pi}"] = _wrap16(r0a)
            d[f"r0b_p{pi}"] = _wrap16(r0b)
            # edge rounds
            ge = np.full((NR[pi], 128), PAD_REL, dtype=np.int64)
            pos = 0
            for w in range(NWIN):
                nodes = grid[w]
                nv = nodes >= 0
                nn = np.where(nv, nodes, 0)
                cw = np.where(nv, cnt[nn], 0)          # [128]
                Rw = int(RW[pi][w])
                if Rw == 0:
                    continue
                rows = np.arange(Rw)[:, None]           # [Rw, 1]
                emask = rows < cw[None, :]              # [Rw, 128]
                eidx = starts[nn][None, :] + rows
                eidx = np.minimum(eidx, len(s_sorted) - 1 if len(s_sorted) else 0)
                if len(s_sorted):
                    vals = trow[s_sorted[eidx]] - half_base
                    ge[pos:pos + Rw][emask] = vals[emask]
                pos += Rw
            d[f"ge_p{pi}"] = _wrap16(ge.reshape(-1))
            # scatter idx: canonical local node id per slot
            sc = np.where(gv >= 0, gv, DUMMY_NODE)
            d[f"sc_p{pi}"] = _wrap16(sc)
        # pooling
        g_lo = int(batch[c * SH])
        g_hi = int(batch[min((c + 1) * SH, N) - 1])
        assert g_hi - g_lo < 256, f"graph span too large: {g_hi - g_lo}"
        nodeid = c * SH + np.arange(WSLOT)
        gid = np.where(nodeid < (c + 1) * SH, batch[np.minimum(nodeid, N - 1)] - g_lo,
                       10 ** 9).astype(np.float32)
        d["gid"] = gid.reshape(NWIN, 128).T.copy()  # [128, NWIN]
        cntg = np.bincount(batch, minlength=G)
        for li in (1, 2):
            ps = np.zeros(256, dtype=np.int64)
            for k in (0, 1):
                gg = g_lo + k * 128 + np.arange(128)
                ok = gg < G
                ps[k * 128:(k + 1) * 128] = np.where(ok, (li - 1) * G + gg, EMB_DUMMY)
            d[f"psx{li}"] = _wrap16(ps)
        rcp = np.zeros((128, 2), dtype=np.float32)
        for k in (0, 1):
            gg = g_lo + k * 128 + np.arange(128)
            ok = gg < G
            rcp[:, k] = np.where(ok, 1.0 / np.maximum(cntg[np.minimum(gg, G - 1)], 1), 0.0)
        d["rcp"] = rcp
        percore.append(d)

    meta = dict(RW=RW, NR=NR, percore=percore, cores=cores)
    return meta


# ---------------- device program ----------------

def _build_program(RW, NR):
    import concourse.bacc as bacc
    import concourse.bass as bass
    import concourse.mybir as mybir
    import concourse.tile as tile
    from concourse.masks import make_identity

    f32 = mybir.dt.float32
    bf16 = mybir.dt.bfloat16
    i16 = mybir.dt.int16
    AF = mybir.ActivationFunctionType
    ALU = mybir.AluOpType

    import concourse.tile_sem_assignment as tsa
    import concourse.bass_isa as bass_isa
    if not getattr(tsa, "_qaware_patched", False):
        _orig_assign = tsa.TileClockTick._assign_tick

        def _assign_tick_qaware(self, inst):
            if (isinstance(inst, tsa.DMAInst)
                    and inst.engine == mybir.EngineType.Pool
                    and not isinstance(inst, bass_isa.UserSyncedRemoteDMADescs)):
                q = int(getattr(inst, "queue_num", 0) or 0)
                cnt = getattr(self, "_q_lane_cnt", None)
                if cnt is None:
                    cnt = self._q_lane_cnt = [0] * 8
                lane = (2 * q + (cnt[q] & 1)) % self.swdge_sem_count
                cnt[q] ^= 1
                self.next_sw_dma_idx = lane
            return _orig_assign(self, inst)

        tsa.TileClockTick._assign_tick = _assign_tick_qaware
        tsa._qaware_patched = True

    _nocc = bool(int(os.environ.get("K_NOCC", "0")))
    nc = bacc.Bacc("TRN2", target_bir_lowering=False, debug=False,
                   num_devices=1 if _nocc else NC,
                   num_swdge_queues=4)

    # ---- I/O ----
    xs = nc.dram_tensor("xs", [WSLOT, D], f32, kind="ExternalInput")
    Wd, asd, add_, bd = {}, {}, {}, {}
    Qd, QTd, qsd = {}, {}, {}
    for li in (1, 2):
        Wd[li] = nc.dram_tensor(f"W{li}", [D, D], f32, kind="ExternalInput")
        Qd[li] = nc.dram_tensor(f"Q{li}", [D, D], f32, kind="ExternalInput")
        QTd[li] = nc.dram_tensor(f"QT{li}", [D, D], f32, kind="ExternalInput")
        qsd[li] = nc.dram_tensor(f"qs{li}", [128, 4], f32, kind="ExternalInput")
        asd[li] = nc.dram_tensor(f"asrc{li}", [1, D], f32, kind="ExternalInput")
        add_[li] = nc.dram_tensor(f"adst{li}", [1, D], f32, kind="ExternalInput")
        bd[li] = nc.dram_tensor(f"b{li}", [1, D], f32, kind="ExternalInput")
    iota2 = nc.dram_tensor("iota2", [2, D], f32, kind="ExternalInput")
    r0d, ged, scd = {}, {}, {}
    for pi in (0, 1):
        r0d[(pi, 0)] = nc.dram_tensor(f"r0a_p{pi}", [128, WSLOT // 16], i16, kind="ExternalInput")
        r0d[(pi, 1)] = nc.dram_tensor(f"r0b_p{pi}", [128, WSLOT // 16], i16, kind="ExternalInput")
        ged[pi] = nc.dram_tensor(f"ge_p{pi}", [128, NR[pi] * 8], i16, kind="ExternalInput")
        scd[pi] = nc.dram_tensor(f"sc_p{pi}", [128, WSLOT // 16], i16, kind="ExternalInput")
    gidd = nc.dram_tensor("gid", [128, NWIN], f32, kind="ExternalInput")
    psxd = {li: nc.dram_tensor(f"psx{li}", [128, 16], i16, kind="ExternalInput") for li in (1, 2)}
    rcpd = nc.dram_tensor("rcp", [128, 2], f32, kind="ExternalInput")
    out_emb = nc.dram_tensor("out", [2 * G, D], f32, kind="ExternalOutput")

    # ---- internal DRAM ----
    h_own = {li: nc.dram_tensor(f"h_own{li}", [BLK, D], bf16, kind="Internal") for li in (1, 2)}
    table = {li: nc.dram_tensor(f"table{li}", [TBL, D], bf16, kind="Internal", addr_space="Shared") for li in (1, 2)}
    partial = {li: nc.dram_tensor(f"partial{li}", [6400, 192], f32, kind="Internal") for li in (1, 2)}
    emb_local = nc.dram_tensor("emb_local", [EMB_ROWS, D], f32, kind="Internal")
    emb_red = nc.dram_tensor("emb_red", [EMB_ROWS, D], f32, kind="Internal", addr_space="Shared")

    with tile.TileContext(nc) as tc:
        with (
            tc.tile_pool(name="const", bufs=1) as cpool,
            tc.tile_pool(name="sb", bufs=3) as sb,
            tc.tile_pool(name="sb3", bufs=4) as sb3,
            tc.tile_pool(name="psum", bufs=2, space="PSUM") as pp,
            tc.tile_pool(name="psum1", bufs=1, space="PSUM") as pp1,
        ):
            # ---------- constants ----------
            ident = cpool.tile([128, 128], bf16, tag="ident")
            make_identity(nc, ident[:])
            Wb, QTb, qss = {}, {}, {}
            for li in (1, 2):
                wf = sb.tile([128, 128], f32, tag="wf")
                nc.sync.dma_start(wf[:], Wd[li][:, :])
                wbt = sb.tile([128, 128], bf16, tag="wbt")
                nc.vector.tensor_copy(wbt[:], wf[:])
                qf = sb.tile([128, 128], f32, tag="wf")
                nc.sync.dma_start(qf[:], Qd[li][:, :])
                qb = sb.tile([128, 128], bf16, tag="qb")
                nc.vector.tensor_copy(qb[:], qf[:])
                # Wq = W @ Q  (lhsT = W^T via PE transpose)
                wtp = pp.tile([128, 128], bf16, tag="mm")
                nc.tensor.transpose(wtp[:], wbt[:], ident[:])
                wts = sb.tile([128, 128], bf16, tag="wts")
                nc.vector.tensor_copy(wts[:], wtp[:])
                wqp = pp.tile([128, 128], f32, tag="mm")
                nc.tensor.matmul(wqp[:], lhsT=wts[:], rhs=qb[:], start=True, stop=True)
                Wb[li] = cpool.tile([128, 128], bf16, tag=f"W{li}b", name=f"Wb{li}")
                nc.vector.tensor_copy(Wb[li][:], wqp[:])
                qtf = sb.tile([128, 128], f32, tag="wf")
                nc.sync.dma_start(qtf[:], QTd[li][:, :])
                QTb[li] = cpool.tile([128, 128], bf16, tag=f"QT{li}b", name=f"QTb{li}")
                nc.vector.tensor_copy(QTb[li][:], qtf[:])
                qst = cpool.tile([128, 4], f32, tag=f"qs{li}", name=f"qss{li}")
                nc.sync.dma_start(qst[:], qsd[li][:, :])
                qss[li] = qst
            # per-layer [1,128] vectors
            vrow = {}
            for li in (1, 2):
                t = cpool.tile([1, 128], f32, tag=f"vsrc{li}", name=f"vsrc{li}")
                nc.sync.dma_start(t[:], asd[li][:, :])
                vrow[("asrc", li)] = t
                t = cpool.tile([1, 128], f32, tag=f"vdst{li}", name=f"vdst{li}")
                nc.sync.dma_start(t[:], add_[li][:, :])
                vrow[("adst", li)] = t
                t = cpool.tile([1, 128], f32, tag=f"vb{li}", name=f"vb{li}")
                nc.sync.dma_start(t[:], bd[li][:, :])
                vrow[("b", li)] = t
            iot0 = cpool.tile([1, 128], f32, tag="iota0")
            nc.sync.dma_start(iot0[:], iota2[0:1, :])
            iot1 = cpool.tile([1, 128], f32, tag="iota1")
            nc.sync.dma_start(iot1[:], iota2[1:2, :])
            iot_rep0 = cpool.tile([128, 128], f32, tag="iotr0")
            nc.gpsimd.partition_broadcast(iot_rep0[:], iot0[:])
            iot_rep1 = cpool.tile([128, 128], f32, tag="iotr1")
            nc.gpsimd.partition_broadcast(iot_rep1[:], iot1[:])
            # gather/scatter idx tiles
            r0sb, gesb, scsb = {}, {}, {}
            for pi in (0, 1):
                for h in (0, 1):
                    t = cpool.tile([128, WSLOT // 16], i16, tag=f"r0_{pi}_{h}", name=f"r0sb{pi}{h}")
                    nc.sync.dma_start(t[:], r0d[(pi, h)][:, :])
                    r0sb[(pi, h)] = t
                t = cpool.tile([128, NR[pi] * 8], i16, tag=f"ge{pi}", name=f"gesb{pi}")
                nc.sync.dma_start(t[:], ged[pi][:, :])
                gesb[pi] = t
                t = cpool.tile([128, WSLOT // 16], i16, tag=f"sc{pi}", name=f"scsb{pi}")
                nc.sync.dma_start(t[:], scd[pi][:, :])
                scsb[pi] = t
            gids = cpool.tile([128, NWIN], f32, tag="gid")
            nc.sync.dma_start(gids[:], gidd[:, :])
            psxs = {}
            for li in (1, 2):
                t = cpool.tile([128, 16], i16, tag=f"psx{li}", name=f"psxsb{li}")
                nc.sync.dma_start(t[:], psxd[li][:, :])
                psxs[li] = t
            rcps = cpool.tile([128, 2], f32, tag="rcp")
            nc.sync.dma_start(rcps[:], rcpd[:, :])
            # persistent h1 (bf16) for layer 2
            h1_sb = cpool.tile([128, WSLOT], bf16, tag="h1keep")

            # ---------- zero internal DRAM ----------
            zz = cpool.tile([128, 1536], f32, tag="zz")
            nc.vector.memset(zz[:], 0.0)

            def zero_bufs():
                for li in (1, 2):
                    flat = partial[li].ap().rearrange("a b -> (a b)")
                    total = 6400 * 192
                    step = 128 * 1536
                    off = 0
                    while off < total:
                        n = min(step, total - off)
                        nc.sync.dma_start(flat[off:off + n].rearrange("(p f) -> p f", p=128),
                                          zz[:, : n // 128])
                        off += n
                flat = emb_local.ap().rearrange("a b -> (a b)")
                total = EMB_ROWS * D
                off = 0
                while off < total:
                    n = min(128 * 1536, total - off)
                    nc.sync.dma_start(flat[off:off + n].rearrange("(p f) -> p f", p=128),
                                      zz[:, : n // 128])
                    off += n

            _qrr = [0]

            def _nextq():
                q = _qrr[0] % 4
                _qrr[0] += 1
                return q

            # ---------- per-layer ----------
            _phases = os.environ.get("K_PHASES", "tanp")  # t=table, a=agg, n=normalize, p=pool

            def build_layer(li):
                asrc_v = vrow[("asrc", li)]
                adst_v = vrow[("adst", li)]
                b_v = vrow[("b", li)]

                # --- table build: h = x @ W (or h1 @ W2) ---
                for t in range(NWIN):
                    if li == 1:
                        xf = sb.tile([128, 128], f32, tag="xf")
                        nc.sync.dma_start(xf[:], xs[t * 128:(t + 1) * 128, :])
                        hb = sb.tile([128, 128], bf16, tag="xb")
                        nc.vector.tensor_copy(hb[:], xf[:])
                        hb_ap = hb[:]
                    else:
                        hb_ap = h1_sb[:, t * 128:(t + 1) * 128]
                    tp = pp.tile([128, 128], bf16, tag="mm")
                    nc.tensor.transpose(tp[:], hb_ap, ident[:])
                    hT = sb.tile([128, 128], bf16, tag="hT")
                    nc.vector.tensor_copy(hT[:], tp[:])
                    hp = pp.tile([128, 128], f32, tag="mm")
                    nc.tensor.matmul(hp[:], lhsT=hT[:], rhs=Wb[li][:], start=True, stop=True)
                    hob = sb.tile([128, 128], bf16, tag="hob")
                    nc.vector.tensor_copy(hob[:], hp[:])
                    nrow = min(128, SH - t * 128)
                    nc.sync.dma_start(h_own[li][t * 128:t * 128 + nrow, :], hob[:nrow, :])
                # special rows
                zrow = sb.tile([1, 128], bf16, tag="zrow")
                nc.vector.memset(zrow[:], 0.0)
                nc.sync.dma_start(h_own[li][SH:SH + 1, :], zrow[:])
                prow = sb.tile([1, 128], bf16, tag="prow")
                nc.vector.memset(prow[:], 0.0)
                nc.vector.tensor_scalar_mul(prow[:, 0:1], qss[li][0:1, 0:1], -PAD_C)
                nc.sync.dma_start(h_own[li][SH + 1:SH + 2, :], prow[:])

                # --- all-gather table ---
                if _nocc:
                    # timing-study mode: fake the AG with a local DMA 8x
                    for r8 in range(NC):
                        nc.sync.dma_start(table[li].ap()[r8 * BLK:(r8 + 1) * BLK, :],
                                          h_own[li].ap()[:, :])
                else:
                    nc.gpsimd.collective_compute(
                        "AllGather", ALU.bypass,
                        replica_groups=[list(range(NC))],
                        ins=[h_own[li].ap()],
                        outs=[table[li].ap()],
                    )

                b_rep = sb.tile([128, 128], f32, tag="brep")
                nc.gpsimd.partition_broadcast(b_rep[:], b_v[:])

                # --- aggregation passes ---
                for pi in ((0, 1) if "a" in _phases else ()):
                    tA = table[li].ap()[0:HALF_ROWS, :]
                    tB = table[li].ap()[HALF_ROWS:2 * HALF_ROWS, :]
                    tP = tA if pi == 0 else tB
                    # round-0 self gather (both halves, merged)
                    g0a = sb.tile([128, NWIN, 128], bf16, tag="g0")
                    g0b = sb.tile([128, NWIN, 128], bf16, tag="g0")
                    for ch in range(0, NWIN, NWIN):
                        cw = min(NWIN, NWIN - ch)
                        ni = cw * 128
                        nc.gpsimd.dma_gather(
                            g0a[:, ch:ch + cw, :], tA,
                            r0sb[(pi, 0)][:, ch * 8:(ch + cw) * 8],
                            ni, ni, 128, queue_num=_nextq(),
                            single_packet=False)
                        nc.gpsimd.dma_gather(
                            g0b[:, ch:ch + cw, :], tB,
                            r0sb[(pi, 1)][:, ch * 8:(ch + cw) * 8],
                            ni, ni, 128, queue_num=_nextq(),
                            single_packet=False)
                    g0m = sb.tile([128, NWIN, 2], bf16, tag="g0m")
                    nc.vector.tensor_tensor(out=g0m[:], in0=g0a[:, :, 0:2],
                                            in1=g0b[:, :, 0:2], op=ALU.add)
                    ta_ = sb.tile([128, NWIN], f32, tag="ta_")
                    nc.vector.tensor_scalar_mul(ta_[:], g0m[:, :, 0], qss[li][:, 1:2])
                    tb_ = sb.tile([128, NWIN], f32, tag="tb_")
                    nc.vector.tensor_scalar_mul(tb_[:], g0m[:, :, 1], qss[li][:, 2:3])
                    adst_all = sb.tile([128, NWIN], f32, tag="adstall")
                    nc.vector.tensor_tensor(out=adst_all[:], in0=ta_[:], in1=tb_[:],
                                            op=ALU.add)

                    # schedule of rounds -> windows
                    rw = RW[pi]
                    sched = []  # (window, r_in_window)
                    for w in range(NWIN):
                        for r in range(int(rw[w])):
                            sched.append((w, r))
                    nrounds = len(sched)
                    assert nrounds == NR[pi]

                    _sub = os.environ.get("K_AGGSUB", "full")
                    # window start offsets in the round sequence
                    wstart = {}
                    _pos = 0
                    for w in range(NWIN):
                        wstart[w] = _pos
                        _pos += int(rw[w])
                    walls = sb.tile([128, max(nrounds, 1)], f32, tag="walls",
                                    name="walls")
                    psw = {}     # window -> psum tile
                    stg = {}     # group -> staging tile
                    ncalls = (nrounds + NB - 1) // NB
                    for k in range(ncalls):
                        lo = k * NB
                        nb = min(NB, nrounds - lo)
                        g = sb3.tile([128, NB, 128], bf16, tag="g")
                        nc.gpsimd.dma_gather(
                            g[:, :nb, :], tP,
                            gesb[pi][:, lo * 8:(lo + nb) * 8],
                            nb * 128, nb * 128, 128,
                            queue_num=_nextq(), single_packet=False)
                        if _sub == "g":
                            continue
                        zt = sb.tile([128, NB], f32, tag="zt")
                        # z = s0 * G[:, :, 0] + adst  (per-window bias)
                        c0 = 0
                        while c0 < nb:
                            w = sched[lo + c0][0]
                            c1 = c0
                            while c1 < nb and sched[lo + c1][0] == w:
                                c1 += 1
                            nc.scalar.activation(zt[:, c0:c1], g[:, c0:c1, 0],
                                                 AF.Identity,
                                                 scale=qss[li][:, 0:1],
                                                 bias=adst_all[:, w:w + 1])
                            c0 = c1
                        # leaky relu: max(z, 0.2*z)
                        zm = sb.tile([128, NB], f32, tag="zm")
                        nc.vector.tensor_scalar_mul(zm[:, :nb], zt[:, :nb], NEG_SLOPE)
                        zl = sb.tile([128, NB], f32, tag="zl")
                        nc.vector.tensor_tensor(out=zl[:, :nb], in0=zt[:, :nb],
                                                in1=zm[:, :nb], op=ALU.max)
                        nc.scalar.activation(walls[:, lo:lo + nb], zl[:, :nb], AF.Exp)
                        if _sub == "gd":
                            continue
                        # scale rows by w: one batched DVE op, contiguous output
                        sg = sb3.tile([128, NB, 128], bf16, tag="sg")
                        nc.vector.tensor_tensor(
                            out=sg[:, :nb, :], in0=g[:, :nb, :],
                            in1=walls[:, lo:lo + nb].rearrange(
                                "p (n o) -> p n o", o=1).to_broadcast([128, nb, 128]),
                            op=ALU.mult)
                        # matmuls
                        if _sub == "gds":
                            continue
                        for j in range(nb):
                            w, r = sched[lo + j]
                            first = (r == 0)
                            last = (r == int(rw[w]) - 1)
                            if first:
                                psw[w] = pp.tile([128, 128], f32, tag="agg", name=f"aggps", bufs=3)
                            nc.tensor.matmul(psw[w][:], lhsT=ident[:],
                                             rhs=sg[:, j, :],
                                             start=first, stop=last)
                            if last:
                                grp = w // WGRP
                                if grp not in stg:
                                    stg[grp] = sb.tile([128, WGRP, 192], f32, tag="stg", name="stg", bufs=3)
                                    nc.vector.memset(stg[grp][:, :, 129:192], 0.0)
                                nc.vector.tensor_copy(stg[grp][:, w % WGRP, 0:128],
                                                      psw[w][:])
                                nc.vector.tensor_reduce(
                                    stg[grp][:, w % WGRP, 128:129],
                                    walls[:, wstart[w]:wstart[w] + int(rw[w])],
                                    axis=mybir.AxisListType.X, op=ALU.add)
                                del psw[w]
                                if (w % WGRP) == WGRP - 1 and _sub != "gdsm":
                                    nc.gpsimd.dma_scatter_add(
                                        partial[li].ap()[:, :],
                                        stg[grp][:].rearrange("p a b -> p (a b)").rearrange(
                                            "p (a b) -> p a b", b=192),
                                        scsb[pi][:, grp * WGRP * 8:(grp + 1) * WGRP * 8],
                                        WGRP * 128, WGRP * 128, 192,
                                        queue_num=_nextq(),
                                        single_packet=False)
                                    del stg[grp]
                    # handle windows with rw == 0 (write zeros for them): none occur
                    # (every node has a self loop so every window has >= 1 round
                    # in one of the passes; zero-round windows in a pass simply
                    # contribute nothing -- partial stays zeroed for them).

                # --- normalize + pool (+ keep h1) ---
                ppool0 = pp1.tile([128, 128], f32, tag="pool0", name=f"pool0_{li}")
                ppool1 = pp1.tile([128, 128], f32, tag="pool1", name=f"pool1_{li}")
                for t in (range(NWIN) if "n" in _phases else ()):
                    pt = sb.tile([128, 192], f32, tag="pt")
                    nc.sync.dma_start(pt[:], partial[li].ap()[t * 128:(t + 1) * 128, :])
                    dn = sb.tile([128, 1], f32, tag="dn")
                    nc.vector.tensor_scalar_max(dn[:], pt[:, 128:129], 1e-30)
                    rc = sb.tile([128, 1], f32, tag="rc")
                    nc.vector.reciprocal(rc[:], dn[:])
                    # un-rotate: numer = numer' @ Q^T
                    nbf = sb.tile([128, 128], bf16, tag="nbf")
                    nc.vector.tensor_copy(nbf[:], pt[:, 0:128])
                    ntp = pp.tile([128, 128], bf16, tag="mm")
                    nc.tensor.transpose(ntp[:], nbf[:], ident[:])
                    nts = sb.tile([128, 128], bf16, tag="nts")
                    nc.vector.tensor_copy(nts[:], ntp[:])
                    unp = pp1.tile([128, 128], f32, tag="un", name="unps")
                    nc.tensor.matmul(unp[:], lhsT=nts[:], rhs=QTb[li][:],
                                     start=True, stop=True)
                    hn = sb.tile([128, 128], f32, tag="hn")
                    nc.vector.tensor_scalar_mul(hn[:], unp[:], rc[:])
                    hb2 = sb.tile([128, 128], f32, tag="hb2")
                    nc.vector.tensor_tensor(out=hb2[:], in0=hn[:],
                                            in1=b_rep[:], op=ALU.add)
                    hr = sb.tile([128, 128], f32, tag="hr")
                    nc.scalar.activation(hr[:], hb2[:], AF.Relu)
                    oh = sb.tile([128, 128], f32, tag="oh")
                    nc.vector.tensor_tensor(
                        out=oh[:], in0=gids[:, t:t + 1].to_broadcast([128, 128]),
                        in1=iot_rep0[:], op=ALU.is_equal)
                    nc.tensor.matmul(ppool0[:], lhsT=oh[:], rhs=hr[:],
                                     start=(t == 0), stop=(t == NWIN - 1))
                    oh2 = sb.tile([128, 128], f32, tag="oh2")
                    nc.vector.tensor_tensor(
                        out=oh2[:], in0=gids[:, t:t + 1].to_broadcast([128, 128]),
                        in1=iot_rep1[:], op=ALU.is_equal)
                    nc.tensor.matmul(ppool1[:], lhsT=oh2[:], rhs=hr[:],
                                     start=(t == 0), stop=(t == NWIN - 1))
                    if li == 1:
                        nc.vector.tensor_copy(h1_sb[:, t * 128:(t + 1) * 128], hr[:])
                # emb partials
                for k, ppx in (((0, ppool0), (1, ppool1)) if "n" in _phases else ()):
                    em = sb.tile([128, 1, 128], f32, tag="em")
                    nc.vector.tensor_scalar_mul(em[:, 0, :], ppx[:], rcps[:, k:k + 1])
                    nc.gpsimd.dma_scatter_add(
                        emb_local.ap()[:, :], em[:],
                        psxs[li][:, k * 8:(k + 1) * 8],
                        128, 128, 128, queue_num=_nextq())

            _stages = int(os.environ.get("K_STAGES", "3"))
            _repeat = int(os.environ.get("K_REPEAT", "1"))
            for _rep in range(_repeat):
                zero_bufs()
                if _stages >= 1:
                    build_layer(1)
                if _stages >= 2:
                    build_layer(2)
                # --- final AllReduce ---
                if _nocc:
                    nc.sync.dma_start(emb_red.ap()[:, :], emb_local.ap()[:, :])
                else:
                    nc.gpsimd.collective_compute(
                        "AllReduce", ALU.add,
                        replica_groups=[list(range(NC))],
                        ins=[emb_local.ap()],
                        outs=[emb_red.ap()],
                    )
            for t in range(4):
                ot = sb.tile([128, 512], f32, tag="ot")
                nc.sync.dma_start(
                    ot[:], emb_red.ap()[t * 512:(t + 1) * 512, :].rearrange(
                        "(a b) d -> a (b d)", a=128))
                nc.sync.dma_start(
                    out_emb[t * 512:(t + 1) * 512, :].rearrange(
                        "(a b) d -> a (b d)", a=128), ot[:])

    nc.compile()
    return nc


# ---------------- top-level ----------------

def _make_in_maps(meta, x, W1, att_src1, att_dst1, b1, W2, att_src2, att_dst2, b2):
    x = np.asarray(x, dtype=np.float32)
    def _mk_q(a_src, a_dst):
        a_src = np.asarray(a_src, np.float64).reshape(D)
        a_dst = np.asarray(a_dst, np.float64).reshape(D)
        rng = np.random.default_rng(12345)
        A = np.concatenate([a_src[:, None], a_dst[:, None],
                            rng.standard_normal((D, D - 2))], axis=1)
        Q, R = np.linalg.qr(A)
        Q = Q * np.sign(np.diag(R) + 1e-300)[None, :]
        s0 = float(np.linalg.norm(a_src))
        alpha = float(a_dst @ Q[:, 0])
        beta = float(a_dst @ Q[:, 1])
        qs = np.zeros((128, 4), np.float32)
        qs[:, 0] = s0
        qs[:, 1] = alpha
        qs[:, 2] = beta
        return (Q.astype(np.float32), Q.T.astype(np.float32).copy(), qs)

    Q1, QT1, qs1 = _mk_q(att_src1, att_dst1)
    Q2, QT2, qs2 = _mk_q(att_src2, att_dst2)
    common = {
        "W1": np.asarray(W1, np.float32), "W2": np.asarray(W2, np.float32),
        "Q1": Q1, "QT1": QT1, "qs1": qs1,
        "Q2": Q2, "QT2": QT2, "qs2": qs2,
        "asrc1": np.asarray(att_src1, np.float32).reshape(1, D),
        "adst1": np.asarray(att_dst1, np.float32).reshape(1, D),
        "b1": np.asarray(b1, np.float32).reshape(1, D),
        "asrc2": np.asarray(att_src2, np.float32).reshape(1, D),
        "adst2": np.asarray(att_dst2, np.float32).reshape(1, D),
        "b2": np.asarray(b2, np.float32).reshape(1, D),
        "iota2": np.stack([np.arange(128), 128 + np.arange(128)]).astype(np.float32),
    }
    in_maps = []
    for c in range(NC):
        d = dict(common)
        xp = np.zeros((WSLOT, D), np.float32)
        xp[:SH] = x[c * SH:(c + 1) * SH]
        d["xs"] = xp
        pc = meta["percore"][c]
        for pi in (0, 1):
            d[f"r0a_p{pi}"] = pc[f"r0a_p{pi}"]
            d[f"r0b_p{pi}"] = pc[f"r0b_p{pi}"]
            d[f"ge_p{pi}"] = pc[f"ge_p{pi}"]
            d[f"sc_p{pi}"] = pc[f"sc_p{pi}"]
        d["gid"] = pc["gid"]
        d["psx1"] = pc["psx1"]
        d["psx2"] = pc["psx2"]
        d["rcp"] = pc["rcp"]
        in_maps.append(d)
    return in_maps


def kernel(x, edge_index, batch, W1, att_src1, att_dst1, b1,
           W2, att_src2, att_dst2, b2, _trace=False):
    if "/opt/trn_rl_repo" not in sys.path:
        sys.path.insert(0, "/opt/trn_rl_repo")
    from concourse.bass_utils import run_bass_kernel_spmd

    meta = host_prep(edge_index, batch)
    key = (tuple(meta["NR"]), tuple(meta["RW"][0].tolist()), tuple(meta["RW"][1].tolist()))
    if key not in _CACHE:
        _CACHE[key] = _build_program(meta["RW"], meta["NR"])
    nc = _CACHE[key]

    in_maps = _make_in_maps(meta, x, W1, att_src1, att_dst1, b1,
                            W2, att_src2, att_dst2, b2)
    res = run_bass_kernel_spmd(nc, in_maps, core_ids=list(range(NC)),
                               trace=_trace)
    out = res.results[0]["out"]
    emb1 = np.asarray(out[:G], np.float32).copy()
    emb2 = np.asarray(out[G:2 * G], np.float32).copy()
    if _trace:
        kernel._last_results = res
    return (emb1, emb2)



# revision 1
# speedup vs baseline: 5.4080x; 5.4080x over previous
"""GAT 2-layer + global mean pool, distributed over 8 TRN2 NeuronCores.

Sharding: 8 dst-shards of 6250 nodes. Per layer:
  - each core computes rotated features h' = x @ (W @ Q) for its shard
    (bf16), where Q is orthonormal with q0 ~ a_src and span(q0,q1) covering
    a_dst, so asrc = s0*h'[0] and adst = alpha*h'[0] + beta*h'[1] are free
    columns of every gathered row; AllGather -> full 50016-row table
  - edges split into two passes by src half (int16 gather idx < 32767)
  - per pass: degree-sorted windows of 128 dst nodes, padded rounds;
    dma_gather (4096 idxs/call, single_packet=False) pulls h'[src] rows;
    ACT computes z = s0*G[:,:,0] + adst (per-window bias), DVE lrelu,
    ACT exp -> w; one batched DVE mul scales rows by w;
    PE accumulates numerators per window via identity-lhsT matmuls (PSUM),
    DVE reduces w for denominators;
    dma_scatter_add canonicalizes [numer'|denom] into a DRAM partial buffer
  - normalize: un-rotate numer = numer' @ Q^T (PE), reciprocal, bias, relu;
    one-hot matmul pooling; final AllReduce of pooled embeddings.
"""

import os
import sys

import numpy as np

# ---------------- constants (hardcoded problem shape) ----------------
N = 50000
E = 600000
G = 1024
D = 128
NEG_SLOPE = 0.2
NC = 8
SH = N // NC          # 6250 nodes per shard
HB = N // 2           # 25000 per src half
BLK = SH + 2          # block rows in table: shard + zero row + pad row
TBL = NC * BLK        # 50016
HALF_ROWS = 4 * BLK   # 25008 rows per half
NWIN = (SH + 127) // 128   # 49 windows
WSLOT = NWIN * 128         # 6272 slots
ZERO_REL = SH              # zero row, relative to block 0 of a half
PAD_REL = SH + 1           # pad row (h = -C*a_src)
NB = 32                    # rounds per gather call (single_packet=False lifts the 1024-idx limit)
WGRP = 7                   # windows per scatter group
PAD_C = 30000.0            # pad row scale -> logit ~ -30000*||a||^2
DUMMY_NODE = SH            # scatter dummy local row
EMB_ROWS = 2176            # 2*1024 + dummy + pad to 128 multiple
EMB_DUMMY = 2048

_CACHE = {}


def _table_row(n):
    return (n // SH) * BLK + (n % SH)


def _wrap16(a):
    """int array -> [128, len/16] int16 wrapped + replicated layout."""
    a = np.asarray(a, dtype=np.int16)
    L = a.shape[0]
    assert L % 16 == 0
    w = a.reshape(L // 16, 16).T  # [16, L/16]
    return np.tile(w, (8, 1)).copy()  # [128, L/16]


def host_prep(edge_index, batch):
    """Pure index preprocessing. Returns per-core input dicts pieces + meta."""
    src = np.concatenate([np.asarray(edge_index[0]), np.arange(N)]).astype(np.int64)
    dst = np.concatenate([np.asarray(edge_index[1]), np.arange(N)]).astype(np.int64)
    batch = np.asarray(batch).astype(np.int64)

    trow = _table_row(np.arange(N))  # table row of each node

    cores = []
    for c in range(NC):
        sel = (dst // SH) == c
        s_c = src[sel]
        d_c = dst[sel] - c * SH
        passes = []
        for pi in (0, 1):
            m = (s_c // HB) == pi
            s = s_c[m]
            dl = d_c[m]
            cnt = np.bincount(dl, minlength=SH)
            # CSR by local dst
            eorder = np.argsort(dl, kind="stable")
            s_sorted = s[eorder]
            starts = np.zeros(SH + 1, dtype=np.int64)
            starts[1:] = np.cumsum(cnt)
            perm = np.argsort(-cnt, kind="stable")  # nodes by degree desc
            rw = np.zeros(NWIN, dtype=np.int64)
            for w in range(NWIN):
                nodes = perm[w * 128:(w + 1) * 128]
                if len(nodes):
                    rw[w] = cnt[nodes].max()
            passes.append(dict(cnt=cnt, s_sorted=s_sorted, starts=starts,
                               perm=perm, rw=rw))
        cores.append(passes)

    # uniform window profiles across cores (per pass)
    RW = []
    for pi in (0, 1):
        rw = np.zeros(NWIN, dtype=np.int64)
        for c in range(NC):
            rw = np.maximum(rw, cores[c][pi]["rw"])
        RW.append(rw)
    NR = [int(RW[0].sum()), int(RW[1].sum())]

    # per-core index arrays
    percore = []
    for c in range(NC):
        d = {}
        for pi in (0, 1):
            P = cores[c][pi]
            perm, cnt, starts, s_sorted = P["perm"], P["cnt"], P["starts"], P["s_sorted"]
            half_base = pi * HALF_ROWS
            # node grid [NWIN, 128], -1 for pad slots
            grid = np.full((NWIN, 128), -1, dtype=np.int64)
            flat = perm
            grid.reshape(-1)[: len(flat)] = flat
            # round-0 self-gather idx, both halves
            r0a = np.full(WSLOT, ZERO_REL, dtype=np.int64)
            r0b = np.full(WSLOT, ZERO_REL, dtype=np.int64)
            gv = grid.reshape(-1)
            valid = gv >= 0
            gl = np.where(valid, gv + c * SH, 0)   # canonical node id
            isA = valid & (gl // HB == 0)
            isB = valid & (gl // HB == 1)
            r0a[isA] = trow[gl[isA]]
            r0b[isB] = trow[gl[isB]] - HALF_ROWS
            d[f"r0a_p{pi}"] = _wrap16(r0a)
            d[f"r0b_p{pi}"] = _wrap16(r0b)
            # edge rounds
            ge = np.full((NR[pi], 128), PAD_REL, dtype=np.int64)
            pos = 0
            for w in range(NWIN):
                nodes = grid[w]
                nv = nodes >= 0
                nn = np.where(nv, nodes, 0)
                cw = np.where(nv, cnt[nn], 0)          # [128]
                Rw = int(RW[pi][w])
                if Rw == 0:
                    continue
                rows = np.arange(Rw)[:, None]           # [Rw, 1]
                emask = rows < cw[None, :]              # [Rw, 128]
                eidx = starts[nn][None, :] + rows
                eidx = np.minimum(eidx, len(s_sorted) - 1 if len(s_sorted) else 0)
                if len(s_sorted):
                    vals = trow[s_sorted[eidx]] - half_base
                    ge[pos:pos + Rw][emask] = vals[emask]
                pos += Rw
            d[f"ge_p{pi}"] = _wrap16(ge.reshape(-1))
            # scatter idx: canonical local node id per slot
            sc = np.where(gv >= 0, gv, DUMMY_NODE)
            d[f"sc_p{pi}"] = _wrap16(sc)
        # pooling
        g_lo = int(batch[c * SH])
        g_hi = int(batch[min((c + 1) * SH, N) - 1])
        assert g_hi - g_lo < 256, f"graph span too large: {g_hi - g_lo}"
        nodeid = c * SH + np.arange(WSLOT)
        gid = np.where(nodeid < (c + 1) * SH, batch[np.minimum(nodeid, N - 1)] - g_lo,
                       10 ** 9).astype(np.float32)
        d["gid"] = gid.reshape(NWIN, 128).T.copy()  # [128, NWIN]
        cntg = np.bincount(batch, minlength=G)
        for li in (1, 2):
            ps = np.zeros(256, dtype=np.int64)
            for k in (0, 1):
                gg = g_lo + k * 128 + np.arange(128)
                ok = gg < G
                ps[k * 128:(k + 1) * 128] = np.where(ok, (li - 1) * G + gg, EMB_DUMMY)
            d[f"psx{li}"] = _wrap16(ps)
        rcp = np.zeros((128, 2), dtype=np.float32)
        for k in (0, 1):
            gg = g_lo + k * 128 + np.arange(128)
            ok = gg < G
            rcp[:, k] = np.where(ok, 1.0 / np.maximum(cntg[np.minimum(gg, G - 1)], 1), 0.0)
        d["rcp"] = rcp
        percore.append(d)

    meta = dict(RW=RW, NR=NR, percore=percore, cores=cores)
    return meta


# ---------------- device program ----------------

def _build_program(RW, NR):
    import concourse.bacc as bacc
    import concourse.bass as bass
    import concourse.mybir as mybir
    import concourse.tile as tile
    from concourse.masks import make_identity

    f32 = mybir.dt.float32
    bf16 = mybir.dt.bfloat16
    i16 = mybir.dt.int16
    AF = mybir.ActivationFunctionType
    ALU = mybir.AluOpType

    import concourse.tile_sem_assignment as tsa
    import concourse.bass_isa as bass_isa
    if not getattr(tsa, "_qaware_patched", False):
        _orig_assign = tsa.TileClockTick._assign_tick

        def _assign_tick_qaware(self, inst):
            if (isinstance(inst, tsa.DMAInst)
                    and inst.engine == mybir.EngineType.Pool
                    and not isinstance(inst, bass_isa.UserSyncedRemoteDMADescs)):
                q = int(getattr(inst, "queue_num", 0) or 0)
                cnt = getattr(self, "_q_lane_cnt", None)
                if cnt is None:
                    cnt = self._q_lane_cnt = [0] * 8
                lane = (2 * q + (cnt[q] & 1)) % self.swdge_sem_count
                cnt[q] ^= 1
                self.next_sw_dma_idx = lane
            return _orig_assign(self, inst)

        tsa.TileClockTick._assign_tick = _assign_tick_qaware
        tsa._qaware_patched = True

    _nocc = bool(int(os.environ.get("K_NOCC", "0")))
    nc = bacc.Bacc("TRN2", target_bir_lowering=False, debug=False,
                   num_devices=1 if _nocc else NC,
                   num_swdge_queues=4)

    # ---- I/O ----
    xs = nc.dram_tensor("xs", [WSLOT, D], f32, kind="ExternalInput")
    Wd, asd, add_, bd = {}, {}, {}, {}
    Qd, QTd, qsd = {}, {}, {}
    for li in (1, 2):
        Wd[li] = nc.dram_tensor(f"W{li}", [D, D], f32, kind="ExternalInput")
        Qd[li] = nc.dram_tensor(f"Q{li}", [D, D], f32, kind="ExternalInput")
        QTd[li] = nc.dram_tensor(f"QT{li}", [D, D], f32, kind="ExternalInput")
        qsd[li] = nc.dram_tensor(f"qs{li}", [128, 4], f32, kind="ExternalInput")
        asd[li] = nc.dram_tensor(f"asrc{li}", [1, D], f32, kind="ExternalInput")
        add_[li] = nc.dram_tensor(f"adst{li}", [1, D], f32, kind="ExternalInput")
        bd[li] = nc.dram_tensor(f"b{li}", [1, D], f32, kind="ExternalInput")
    iota2 = nc.dram_tensor("iota2", [2, D], f32, kind="ExternalInput")
    r0d, ged, scd = {}, {}, {}
    for pi in (0, 1):
        r0d[(pi, 0)] = nc.dram_tensor(f"r0a_p{pi}", [128, WSLOT // 16], i16, kind="ExternalInput")
        r0d[(pi, 1)] = nc.dram_tensor(f"r0b_p{pi}", [128, WSLOT // 16], i16, kind="ExternalInput")
        ged[pi] = nc.dram_tensor(f"ge_p{pi}", [128, NR[pi] * 8], i16, kind="ExternalInput")
        scd[pi] = nc.dram_tensor(f"sc_p{pi}", [128, WSLOT // 16], i16, kind="ExternalInput")
    gidd = nc.dram_tensor("gid", [128, NWIN], f32, kind="ExternalInput")
    psxd = {li: nc.dram_tensor(f"psx{li}", [128, 16], i16, kind="ExternalInput") for li in (1, 2)}
    rcpd = nc.dram_tensor("rcp", [128, 2], f32, kind="ExternalInput")
    out_emb = nc.dram_tensor("out", [2 * G, D], f32, kind="ExternalOutput")

    # ---- internal DRAM ----
    h_own = {li: nc.dram_tensor(f"h_own{li}", [BLK, D], bf16, kind="Internal") for li in (1, 2)}
    table = {li: nc.dram_tensor(f"table{li}", [TBL, D], bf16, kind="Internal", addr_space="Shared") for li in (1, 2)}
    partial = {li: nc.dram_tensor(f"partial{li}", [6400, 192], f32, kind="Internal") for li in (1, 2)}
    emb_local = nc.dram_tensor("emb_local", [EMB_ROWS, D], f32, kind="Internal")
    emb_red = nc.dram_tensor("emb_red", [EMB_ROWS, D], f32, kind="Internal", addr_space="Shared")

    with tile.TileContext(nc) as tc:
        with (
            tc.tile_pool(name="const", bufs=1) as cpool,
            tc.tile_pool(name="sb", bufs=3) as sb,
            tc.tile_pool(name="sb3", bufs=4) as sb3,
            tc.tile_pool(name="psum", bufs=2, space="PSUM") as pp,
            tc.tile_pool(name="psum1", bufs=1, space="PSUM") as pp1,
        ):
            # ---------- constants ----------
            ident = cpool.tile([128, 128], bf16, tag="ident")
            make_identity(nc, ident[:])
            Wb, QTb, qss = {}, {}, {}
            for li in (1, 2):
                wf = sb.tile([128, 128], f32, tag="wf")
                nc.sync.dma_start(wf[:], Wd[li][:, :])
                wbt = sb.tile([128, 128], bf16, tag="wbt")
                nc.vector.tensor_copy(wbt[:], wf[:])
                qf = sb.tile([128, 128], f32, tag="wf")
                nc.sync.dma_start(qf[:], Qd[li][:, :])
                qb = sb.tile([128, 128], bf16, tag="qb")
                nc.vector.tensor_copy(qb[:], qf[:])
                # Wq = W @ Q  (lhsT = W^T via PE transpose)
                wtp = pp.tile([128, 128], bf16, tag="mm")
                nc.tensor.transpose(wtp[:], wbt[:], ident[:])
                wts = sb.tile([128, 128], bf16, tag="wts")
                nc.vector.tensor_copy(wts[:], wtp[:])
                wqp = pp.tile([128, 128], f32, tag="mm")
                nc.tensor.matmul(wqp[:], lhsT=wts[:], rhs=qb[:], start=True, stop=True)
                Wb[li] = cpool.tile([128, 128], bf16, tag=f"W{li}b", name=f"Wb{li}")
                nc.vector.tensor_copy(Wb[li][:], wqp[:])
                qtf = sb.tile([128, 128], f32, tag="wf")
                nc.sync.dma_start(qtf[:], QTd[li][:, :])
                QTb[li] = cpool.tile([128, 128], bf16, tag=f"QT{li}b", name=f"QTb{li}")
                nc.vector.tensor_copy(QTb[li][:], qtf[:])
                qst = cpool.tile([128, 4], f32, tag=f"qs{li}", name=f"qss{li}")
                nc.sync.dma_start(qst[:], qsd[li][:, :])
                qss[li] = qst
            # per-layer [1,128] vectors
            vrow = {}
            for li in (1, 2):
                t = cpool.tile([1, 128], f32, tag=f"vsrc{li}", name=f"vsrc{li}")
                nc.sync.dma_start(t[:], asd[li][:, :])
                vrow[("asrc", li)] = t
                t = cpool.tile([1, 128], f32, tag=f"vdst{li}", name=f"vdst{li}")
                nc.sync.dma_start(t[:], add_[li][:, :])
                vrow[("adst", li)] = t
                t = cpool.tile([1, 128], f32, tag=f"vb{li}", name=f"vb{li}")
                nc.sync.dma_start(t[:], bd[li][:, :])
                vrow[("b", li)] = t
            iot0 = cpool.tile([1, 128], f32, tag="iota0")
            nc.sync.dma_start(iot0[:], iota2[0:1, :])
            iot1 = cpool.tile([1, 128], f32, tag="iota1")
            nc.sync.dma_start(iot1[:], iota2[1:2, :])
            iot_rep0 = cpool.tile([128, 128], f32, tag="iotr0")
            nc.gpsimd.partition_broadcast(iot_rep0[:], iot0[:])
            iot_rep1 = cpool.tile([128, 128], f32, tag="iotr1")
            nc.gpsimd.partition_broadcast(iot_rep1[:], iot1[:])
            # gather/scatter idx tiles
            r0sb, gesb, scsb = {}, {}, {}
            for pi in (0, 1):
                for h in (0, 1):
                    t = cpool.tile([128, WSLOT // 16], i16, tag=f"r0_{pi}_{h}", name=f"r0sb{pi}{h}")
                    nc.sync.dma_start(t[:], r0d[(pi, h)][:, :])
                    r0sb[(pi, h)] = t
                t = cpool.tile([128, NR[pi] * 8], i16, tag=f"ge{pi}", name=f"gesb{pi}")
                nc.sync.dma_start(t[:], ged[pi][:, :])
                gesb[pi] = t
                t = cpool.tile([128, WSLOT // 16], i16, tag=f"sc{pi}", name=f"scsb{pi}")
                nc.sync.dma_start(t[:], scd[pi][:, :])
                scsb[pi] = t
            gids = cpool.tile([128, NWIN], f32, tag="gid")
            nc.sync.dma_start(gids[:], gidd[:, :])
            psxs = {}
            for li in (1, 2):
                t = cpool.tile([128, 16], i16, tag=f"psx{li}", name=f"psxsb{li}")
                nc.sync.dma_start(t[:], psxd[li][:, :])
                psxs[li] = t
            rcps = cpool.tile([128, 2], f32, tag="rcp")
            nc.sync.dma_start(rcps[:], rcpd[:, :])
            # persistent h1 (bf16) for layer 2
            h1_sb = cpool.tile([128, WSLOT], bf16, tag="h1keep")

            # ---------- zero internal DRAM ----------
            zz = cpool.tile([128, 1536], f32, tag="zz")
            nc.vector.memset(zz[:], 0.0)

            def zero_bufs():
                for li in (1, 2):
                    flat = partial[li].ap().rearrange("a b -> (a b)")
                    total = 6400 * 192
                    step = 128 * 1536
                    off = 0
                    while off < total:
                        n = min(step, total - off)
                        nc.sync.dma_start(flat[off:off + n].rearrange("(p f) -> p f", p=128),
                                          zz[:, : n // 128])
                        off += n
                flat = emb_local.ap().rearrange("a b -> (a b)")
                total = EMB_ROWS * D
                off = 0
                while off < total:
                    n = min(128 * 1536, total - off)
                    nc.sync.dma_start(flat[off:off + n].rearrange("(p f) -> p f", p=128),
                                      zz[:, : n // 128])
                    off += n

            _qrr = [0]

            def _nextq():
                q = _qrr[0] % 4
                _qrr[0] += 1
                return q

            # ---------- per-layer ----------
            _phases = os.environ.get("K_PHASES", "tanp")  # t=table, a=agg, n=normalize, p=pool

            def build_layer(li):
                asrc_v = vrow[("asrc", li)]
                adst_v = vrow[("adst", li)]
                b_v = vrow[("b", li)]

                # --- table build: h = x @ W (or h1 @ W2) ---
                for t in range(NWIN):
                    if li == 1:
                        xf = sb.tile([128, 128], f32, tag="xf")
                        nc.sync.dma_start(xf[:], xs[t * 128:(t + 1) * 128, :])
                        hb = sb.tile([128, 128], bf16, tag="xb")
                        nc.vector.tensor_copy(hb[:], xf[:])
                        hb_ap = hb[:]
                    else:
                        hb_ap = h1_sb[:, t * 128:(t + 1) * 128]
                    tp = pp.tile([128, 128], bf16, tag="mm")
                    nc.tensor.transpose(tp[:], hb_ap, ident[:])
                    hT = sb.tile([128, 128], bf16, tag="hT")
                    nc.vector.tensor_copy(hT[:], tp[:])
                    hp = pp.tile([128, 128], f32, tag="mm")
                    nc.tensor.matmul(hp[:], lhsT=hT[:], rhs=Wb[li][:], start=True, stop=True)
                    hob = sb.tile([128, 128], bf16, tag="hob")
                    nc.vector.tensor_copy(hob[:], hp[:])
                    nrow = min(128, SH - t * 128)
                    nc.sync.dma_start(h_own[li][t * 128:t * 128 + nrow, :], hob[:nrow, :])
                # special rows
                zrow = sb.tile([1, 128], bf16, tag="zrow")
                nc.vector.memset(zrow[:], 0.0)
                nc.sync.dma_start(h_own[li][SH:SH + 1, :], zrow[:])
                prow = sb.tile([1, 128], bf16, tag="prow")
                nc.vector.memset(prow[:], 0.0)
                nc.vector.tensor_scalar_mul(prow[:, 0:1], qss[li][0:1, 0:1], -PAD_C)
                nc.sync.dma_start(h_own[li][SH + 1:SH + 2, :], prow[:])

                # --- all-gather table ---
                if _nocc:
                    # timing-study mode: fake the AG with a local DMA 8x
                    for r8 in range(NC):
                        nc.sync.dma_start(table[li].ap()[r8 * BLK:(r8 + 1) * BLK, :],
                                          h_own[li].ap()[:, :])
                else:
                    nc.gpsimd.collective_compute(
                        "AllGather", ALU.bypass,
                        replica_groups=[list(range(NC))],
                        ins=[h_own[li].ap()],
                        outs=[table[li].ap()],
                    )

                b_rep = sb.tile([128, 128], f32, tag="brep")
                nc.gpsimd.partition_broadcast(b_rep[:], b_v[:])

                # --- aggregation passes ---
                for pi in ((0, 1) if "a" in _phases else ()):
                    tA = table[li].ap()[0:HALF_ROWS, :]
                    tB = table[li].ap()[HALF_ROWS:2 * HALF_ROWS, :]
                    tP = tA if pi == 0 else tB
                    # round-0 self gather (both halves, merged)
                    g0a = sb.tile([128, NWIN, 128], bf16, tag="g0")
                    g0b = sb.tile([128, NWIN, 128], bf16, tag="g0")
                    for ch in range(0, NWIN, NWIN):
                        cw = min(NWIN, NWIN - ch)
                        ni = cw * 128
                        nc.gpsimd.dma_gather(
                            g0a[:, ch:ch + cw, :], tA,
                            r0sb[(pi, 0)][:, ch * 8:(ch + cw) * 8],
                            ni, ni, 128, queue_num=_nextq(),
                            single_packet=False)
                        nc.gpsimd.dma_gather(
                            g0b[:, ch:ch + cw, :], tB,
                            r0sb[(pi, 1)][:, ch * 8:(ch + cw) * 8],
                            ni, ni, 128, queue_num=_nextq(),
                            single_packet=False)
                    g0m = sb.tile([128, NWIN, 2], bf16, tag="g0m")
                    nc.vector.tensor_tensor(out=g0m[:], in0=g0a[:, :, 0:2],
                                            in1=g0b[:, :, 0:2], op=ALU.add)
                    ta_ = sb.tile([128, NWIN], f32, tag="ta_")
                    nc.vector.tensor_scalar_mul(ta_[:], g0m[:, :, 0], qss[li][:, 1:2])
                    tb_ = sb.tile([128, NWIN], f32, tag="tb_")
                    nc.vector.tensor_scalar_mul(tb_[:], g0m[:, :, 1], qss[li][:, 2:3])
                    adst_all = sb.tile([128, NWIN], f32, tag="adstall")
                    nc.vector.tensor_tensor(out=adst_all[:], in0=ta_[:], in1=tb_[:],
                                            op=ALU.add)

                    # schedule of rounds -> windows
                    rw = RW[pi]
                    sched = []  # (window, r_in_window)
                    for w in range(NWIN):
                        for r in range(int(rw[w])):
                            sched.append((w, r))
                    nrounds = len(sched)
                    assert nrounds == NR[pi]

                    _sub = os.environ.get("K_AGGSUB", "full")
                    # window start offsets in the round sequence
                    wstart = {}
                    _pos = 0
                    for w in range(NWIN):
                        wstart[w] = _pos
                        _pos += int(rw[w])
                    walls = sb.tile([128, max(nrounds, 1)], f32, tag="walls",
                                    name="walls")
                    psw = {}     # window -> psum tile
                    stg = {}     # group -> staging tile
                    ncalls = (nrounds + NB - 1) // NB
                    for k in range(ncalls):
                        lo = k * NB
                        nb = min(NB, nrounds - lo)
                        g = sb3.tile([128, NB, 128], bf16, tag="g")
                        nc.gpsimd.dma_gather(
                            g[:, :nb, :], tP,
                            gesb[pi][:, lo * 8:(lo + nb) * 8],
                            nb * 128, nb * 128, 128,
                            queue_num=_nextq(), single_packet=False)
                        if _sub == "g":
                            continue
                        zt = sb.tile([128, NB], f32, tag="zt")
                        # z = s0 * G[:, :, 0] + adst  (per-window bias)
                        c0 = 0
                        while c0 < nb:
                            w = sched[lo + c0][0]
                            c1 = c0
                            while c1 < nb and sched[lo + c1][0] == w:
                                c1 += 1
                            nc.scalar.activation(zt[:, c0:c1], g[:, c0:c1, 0],
                                                 AF.Identity,
                                                 scale=qss[li][:, 0:1],
                                                 bias=adst_all[:, w:w + 1])
                            c0 = c1
                        # leaky relu: max(z, 0.2*z)
                        zm = sb.tile([128, NB], f32, tag="zm")
                        nc.vector.tensor_scalar_mul(zm[:, :nb], zt[:, :nb], NEG_SLOPE)
                        zl = sb.tile([128, NB], f32, tag="zl")
                        nc.vector.tensor_tensor(out=zl[:, :nb], in0=zt[:, :nb],
                                                in1=zm[:, :nb], op=ALU.max)
                        nc.scalar.activation(walls[:, lo:lo + nb], zl[:, :nb], AF.Exp)
                        if _sub == "gd":
                            continue
                        # scale rows by w: one batched DVE op, contiguous output
                        sg = sb3.tile([128, NB, 128], bf16, tag="sg")
                        nc.vector.tensor_tensor(
                            out=sg[:, :nb, :], in0=g[:, :nb, :],
                            in1=walls[:, lo:lo + nb].rearrange(
                                "p (n o) -> p n o", o=1).to_broadcast([128, nb, 128]),
                            op=ALU.mult)
                        # matmuls
                        if _sub == "gds":
                            continue
                        for j in range(nb):
                            w, r = sched[lo + j]
                            first = (r == 0)
                            last = (r == int(rw[w]) - 1)
                            if first:
                                psw[w] = pp.tile([128, 128], f32, tag="agg", name=f"aggps", bufs=3)
                            nc.tensor.matmul(psw[w][:], lhsT=ident[:],
                                             rhs=sg[:, j, :],
                                             start=first, stop=last)
                            if last:
                                grp = w // WGRP
                                if grp not in stg:
                                    stg[grp] = sb.tile([128, WGRP, 192], f32, tag="stg", name="stg", bufs=3)
                                    nc.vector.memset(stg[grp][:, :, 129:192], 0.0)
                                nc.vector.tensor_copy(stg[grp][:, w % WGRP, 0:128],
                                                      psw[w][:])
                                nc.vector.tensor_reduce(
                                    stg[grp][:, w % WGRP, 128:129],
                                    walls[:, wstart[w]:wstart[w] + int(rw[w])],
                                    axis=mybir.AxisListType.X, op=ALU.add)
                                del psw[w]
                                if (w % WGRP) == WGRP - 1 and _sub != "gdsm":
                                    nc.gpsimd.dma_scatter_add(
                                        partial[li].ap()[:, :],
                                        stg[grp][:].rearrange("p a b -> p (a b)").rearrange(
                                            "p (a b) -> p a b", b=192),
                                        scsb[pi][:, grp * WGRP * 8:(grp + 1) * WGRP * 8],
                                        WGRP * 128, WGRP * 128, 192,
                                        queue_num=_nextq(),
                                        single_packet=False)
                                    del stg[grp]
                    # handle windows with rw == 0 (write zeros for them): none occur
                    # (every node has a self loop so every window has >= 1 round
                    # in one of the passes; zero-round windows in a pass simply
                    # contribute nothing -- partial stays zeroed for them).

                # --- normalize + pool (+ keep h1) ---
                ppool0 = pp1.tile([128, 128], f32, tag="pool0", name=f"pool0_{li}")
                ppool1 = pp1.tile([128, 128], f32, tag="pool1", name=f"pool1_{li}")
                for t in (range(NWIN) if "n" in _phases else ()):
                    pt = sb.tile([128, 192], f32, tag="pt")
                    nc.sync.dma_start(pt[:], partial[li].ap()[t * 128:(t + 1) * 128, :])
                    dn = sb.tile([128, 1], f32, tag="dn")
                    nc.vector.tensor_scalar_max(dn[:], pt[:, 128:129], 1e-30)
                    rc = sb.tile([128, 1], f32, tag="rc")
                    nc.vector.reciprocal(rc[:], dn[:])
                    # un-rotate: numer = numer' @ Q^T
                    nbf = sb.tile([128, 128], bf16, tag="nbf")
                    nc.vector.tensor_copy(nbf[:], pt[:, 0:128])
                    ntp = pp.tile([128, 128], bf16, tag="mm")
                    nc.tensor.transpose(ntp[:], nbf[:], ident[:])
                    nts = sb.tile([128, 128], bf16, tag="nts")
                    nc.vector.tensor_copy(nts[:], ntp[:])
                    unp = pp1.tile([128, 128], f32, tag="un", name="unps")
                    nc.tensor.matmul(unp[:], lhsT=nts[:], rhs=QTb[li][:],
                                     start=True, stop=True)
                    hn = sb.tile([128, 128], f32, tag="hn")
                    nc.vector.tensor_scalar_mul(hn[:], unp[:], rc[:])
                    hb2 = sb.tile([128, 128], f32, tag="hb2")
                    nc.vector.tensor_tensor(out=hb2[:], in0=hn[:],
                                            in1=b_rep[:], op=ALU.add)
                    hr = sb.tile([128, 128], f32, tag="hr")
                    nc.scalar.activation(hr[:], hb2[:], AF.Relu)
                    oh = sb.tile([128, 128], f32, tag="oh")
                    nc.vector.tensor_tensor(
                        out=oh[:], in0=gids[:, t:t + 1].to_broadcast([128, 128]),
                        in1=iot_rep0[:], op=ALU.is_equal)
                    nc.tensor.matmul(ppool0[:], lhsT=oh[:], rhs=hr[:],
                                     start=(t == 0), stop=(t == NWIN - 1))
                    oh2 = sb.tile([128, 128], f32, tag="oh2")
                    nc.vector.tensor_tensor(
                        out=oh2[:], in0=gids[:, t:t + 1].to_broadcast([128, 128]),
                        in1=iot_rep1[:], op=ALU.is_equal)
                    nc.tensor.matmul(ppool1[:], lhsT=oh2[:], rhs=hr[:],
                                     start=(t == 0), stop=(t == NWIN - 1))
                    if li == 1:
                        nc.vector.tensor_copy(h1_sb[:, t * 128:(t + 1) * 128], hr[:])
                # emb partials
                for k, ppx in (((0, ppool0), (1, ppool1)) if "n" in _phases else ()):
                    em = sb.tile([128, 1, 128], f32, tag="em")
                    nc.vector.tensor_scalar_mul(em[:, 0, :], ppx[:], rcps[:, k:k + 1])
                    nc.gpsimd.dma_scatter_add(
                        emb_local.ap()[:, :], em[:],
                        psxs[li][:, k * 8:(k + 1) * 8],
                        128, 128, 128, queue_num=_nextq())

            _stages = int(os.environ.get("K_STAGES", "3"))
            _repeat = int(os.environ.get("K_REPEAT", "1"))
            for _rep in range(_repeat):
                zero_bufs()
                if _stages >= 1:
                    build_layer(1)
                if _stages >= 2:
                    build_layer(2)
                # --- final AllReduce ---
                if _nocc:
                    nc.sync.dma_start(emb_red.ap()[:, :], emb_local.ap()[:, :])
                else:
                    nc.gpsimd.collective_compute(
                        "AllReduce", ALU.add,
                        replica_groups=[list(range(NC))],
                        ins=[emb_local.ap()],
                        outs=[emb_red.ap()],
                    )
            for t in range(4):
                ot = sb.tile([128, 512], f32, tag="ot")
                nc.sync.dma_start(
                    ot[:], emb_red.ap()[t * 512:(t + 1) * 512, :].rearrange(
                        "(a b) d -> a (b d)", a=128))
                nc.sync.dma_start(
                    out_emb[t * 512:(t + 1) * 512, :].rearrange(
                        "(a b) d -> a (b d)", a=128), ot[:])

    nc.compile()
    return nc


# ---------------- top-level ----------------

def _make_in_maps(meta, x, W1, att_src1, att_dst1, b1, W2, att_src2, att_dst2, b2):
    x = np.asarray(x, dtype=np.float32)
    def _mk_q(a_src, a_dst):
        a_src = np.asarray(a_src, np.float64).reshape(D)
        a_dst = np.asarray(a_dst, np.float64).reshape(D)
        rng = np.random.default_rng(12345)
        A = np.concatenate([a_src[:, None], a_dst[:, None],
                            rng.standard_normal((D, D - 2))], axis=1)
        Q, R = np.linalg.qr(A)
        Q = Q * np.sign(np.diag(R) + 1e-300)[None, :]
        s0 = float(np.linalg.norm(a_src))
        alpha = float(a_dst @ Q[:, 0])
        beta = float(a_dst @ Q[:, 1])
        qs = np.zeros((128, 4), np.float32)
        qs[:, 0] = s0
        qs[:, 1] = alpha
        qs[:, 2] = beta
        return (Q.astype(np.float32), Q.T.astype(np.float32).copy(), qs)

    Q1, QT1, qs1 = _mk_q(att_src1, att_dst1)
    Q2, QT2, qs2 = _mk_q(att_src2, att_dst2)
    common = {
        "W1": np.asarray(W1, np.float32), "W2": np.asarray(W2, np.float32),
        "Q1": Q1, "QT1": QT1, "qs1": qs1,
        "Q2": Q2, "QT2": QT2, "qs2": qs2,
        "asrc1": np.asarray(att_src1, np.float32).reshape(1, D),
        "adst1": np.asarray(att_dst1, np.float32).reshape(1, D),
        "b1": np.asarray(b1, np.float32).reshape(1, D),
        "asrc2": np.asarray(att_src2, np.float32).reshape(1, D),
        "adst2": np.asarray(att_dst2, np.float32).reshape(1, D),
        "b2": np.asarray(b2, np.float32).reshape(1, D),
        "iota2": np.stack([np.arange(128), 128 + np.arange(128)]).astype(np.float32),
    }
    in_maps = []
    for c in range(NC):
        d = dict(common)
        xp = np.zeros((WSLOT, D), np.float32)
        xp[:SH] = x[c * SH:(c + 1) * SH]
        d["xs"] = xp
        pc = meta["percore"][c]
        for pi in (0, 1):
            d[f"r0a_p{pi}"] = pc[f"r0a_p{pi}"]
            d[f"r0b_p{pi}"] = pc[f"r0b_p{pi}"]
            d[f"ge_p{pi}"] = pc[f"ge_p{pi}"]
            d[f"sc_p{pi}"] = pc[f"sc_p{pi}"]
        d["gid"] = pc["gid"]
        d["psx1"] = pc["psx1"]
        d["psx2"] = pc["psx2"]
        d["rcp"] = pc["rcp"]
        in_maps.append(d)
    return in_maps


def kernel(x, edge_index, batch, W1, att_src1, att_dst1, b1,
           W2, att_src2, att_dst2, b2, _trace=False):
    if "/opt/trn_rl_repo" not in sys.path:
        sys.path.insert(0, "/opt/trn_rl_repo")
    from concourse.bass_utils import run_bass_kernel_spmd

    meta = host_prep(edge_index, batch)
    key = (tuple(meta["NR"]), tuple(meta["RW"][0].tolist()), tuple(meta["RW"][1].tolist()))
    if key not in _CACHE:
        _CACHE[key] = _build_program(meta["RW"], meta["NR"])
    nc = _CACHE[key]

    in_maps = _make_in_maps(meta, x, W1, att_src1, att_dst1, b1,
                            W2, att_src2, att_dst2, b2)
    res = run_bass_kernel_spmd(nc, in_maps, core_ids=list(range(NC)),
                               trace=_trace)
    out = res.results[0]["out"]
    emb1 = np.asarray(out[:G], np.float32).copy()
    emb2 = np.asarray(out[G:2 * G], np.float32).copy()
    if _trace:
        kernel._last_results = res
    return (emb1, emb2)

